# revision 11
# baseline (speedup 1.0000x reference)
"""Trainium2 Bass kernel for nn_Decoder (Show-Attend-Tell style LSTM decoder
with additive attention), data-parallel over batch across 8 NeuronCores.

Per-core batch = 8. All matmuls in bf16 with fp32 PSUM accumulation.
Per step, the [WU|Wfb] (6MB), Wih_ctx (16MB) weight blocks and the image
features (8.4MB) stream from HBM double-buffered; W_s and Whh stay
SBUF-resident. All bias terms are folded exactly:
  bW+bU -> stored W_s;  bih+bhh -> precomputed emb-gates;  bv -> softmax no-op;
  bfb -> ones-row matmul;  bh0/bc0 -> ones-row;  bout -> ones-row;
  mean(img)/196 -> folded into Wh0/Wc0 rows.
"""
import numpy as np
import ml_dtypes

import concourse.bass as bass
import concourse.tile as tile
from concourse import bacc, mybir
from concourse.bass_utils import run_bass_kernel_spmd

AF = mybir.ActivationFunctionType
BF16 = mybir.dt.bfloat16
F32 = mybir.dt.float32
I32 = mybir.dt.int32

N_CORES = 8
B = 64
BL = B // N_CORES          # 8 batch rows per core
L = 196                    # image locations
D = 2048                   # image feature dim
RNN = 1024
EMB = 512
T1 = 17                    # SEQ_LEN + 1 steps
VOUT = 10003
START = 10001
NR = T1 * BL               # 136 output rows per core (r = t*8 + b)
BLL = BL * L               # 1568

_cache = {}


def _build_nc():
    nc = bacc.Bacc("TRN2", target_bir_lowering=False, debug=False,
                   num_devices=N_CORES)
    dt = nc.dram_tensor
    inp = {}
    inp["imgT"] = dt("imgT", [128, 16, BLL], BF16, kind="ExternalInput").ap()
    inp["imgl"] = dt("imgl", [128, 2, BL, D], BF16, kind="ExternalInput").ap()
    inp["ww"] = dt("ww", [128, 16, RNN], BF16, kind="ExternalInput").ap()
    inp["bwu"] = dt("bwu", [128, 8], F32, kind="ExternalInput").ap()
    inp["w0"] = dt("w0", [128, 16, 2 * RNN], BF16, kind="ExternalInput").ap()
    inp["b0"] = dt("b0", [1, 2 * RNN], BF16, kind="ExternalInput").ap()
    inp["embt"] = dt("embt", [VOUT, EMB], BF16, kind="ExternalInput").ap()
    inp["tids"] = dt("tids", [NR, 1], I32, kind="ExternalInput").ap()
    inp["wie"] = dt("wie", [128, 4, 4 * RNN], BF16, kind="ExternalInput").ap()
    inp["bg"] = dt("bg", [1, 4 * RNN], BF16, kind="ExternalInput").ap()
    inp["whh"] = dt("whh", [128, 8, 4 * RNN], BF16, kind="ExternalInput").ap()
    inp["vv"] = dt("vv", [128, 8], BF16, kind="ExternalInput").ap()
    inp["wufb"] = dt("wufb", [128, 8, 3 * RNN], BF16, kind="ExternalInput").ap()
    inp["bufb"] = dt("bufb", [1, 3 * RNN], BF16, kind="ExternalInput").ap()
    inp["wic"] = dt("wic", [128, 16, 4 * RNN], BF16, kind="ExternalInput").ap()
    inp["wout"] = dt("wout", [128, 8, VOUT], BF16, kind="ExternalInput").ap()
    inp["bout"] = dt("bout", [1, VOUT], BF16, kind="ExternalInput").ap()
    inp["i8"] = dt("i8", [8, 8], BF16, kind="ExternalInput").ap()
    inp["egd"] = dt("egd", [NR, 4 * RNN], BF16, kind="Internal").ap()
    preds = dt("preds", [NR, VOUT], F32, kind="ExternalOutput").ap()
    alph = dt("alph", [NR, L], F32, kind="ExternalOutput").ap()

    with tile.TileContext(nc) as tc:
        _program(nc, tc, inp, preds, alph)
    nc.compile()
    return nc


def _program(nc, tc, inp, preds, alph):
    mm = nc.tensor.matmul
    act = nc.scalar.activation
    X = mybir.AxisListType.X
    NCH_WS = [(0, 512), (512, 512), (1024, 512), (1536, BLL - 1536)]

    with tc.tile_pool(name="st0", bufs=1) as st0, \
         tc.tile_pool(name="st1", bufs=1) as st1:
        # ------- whole-program state -------
        hT = st0.tile([128, 8, T1 + 1, 16], BF16)   # [k%128, kt, slot, b+pad]
        i8_s = st0.tile([8, 8], BF16)
        ones128 = st0.tile([128, 1], BF16)
        ones1 = st0.tile([1, 128], BF16)
        st_h = st0.tile([16, RNN], BF16)            # dmaT staging, rows 8-15 zero
        nc.vector.memset(hT[:], 0.0)
        nc.vector.memset(ones128[:], 1.0)
        nc.vector.memset(ones1[:], 1.0)
        nc.vector.memset(st_h[:], 0.0)
        nc.sync.dma_start(i8_s[:], inp["i8"][:])

        # ------- state live phases A..C (freed before D) -------
        WsT = st1.tile([128, 8, BLL], BF16)         # [f%128, fchunk, b*196+l]
        cst = st1.tile([BL, RNN], F32)

        # ================= Phase A: W_s, sum_l img, h0/c0 =================
        with tc.tile_pool(name="pA", bufs=1) as pA, \
             tc.tile_pool(name="pAs", bufs=2) as pAs, \
             tc.tile_pool(name="psA", bufs=2, space="PSUM") as psA:
            ww_s = pA.tile([128, 16, RNN], BF16)
            nc.sync.dma_start(ww_s[:], inp["ww"][:])
            bwu_s = pA.tile([128, 8], F32)
            nc.sync.dma_start(bwu_s[:], inp["bwu"][:])
            b0_s = pA.tile([1, 2 * RNN], BF16)
            nc.sync.dma_start(b0_s[:], inp["b0"][:])

            # W_s[f, r] = sum_d WW[d, f] imgT[d, r] + (bW+bU)[f]
            for off, w in NCH_WS:
                imc = pAs.tile([128, 16, 512], BF16, tag="imT")
                nc.sync.dma_start(imc[:, :, 0:w], inp["imgT"][:, :, off:off + w])
                for fc in range(8):
                    ps = psA.tile([128, 512], F32, tag="psws")
                    for kt in range(16):
                        mm(out=ps[:, :w],
                           lhsT=ww_s[:, kt, fc * 128:(fc + 1) * 128],
                           rhs=imc[:, kt, 0:w],
                           start=(kt == 0), stop=(kt == 15))
                    act(WsT[:, fc, off:off + w], ps[:, :w], AF.Identity,
                        bias=bwu_s[:, fc:fc + 1])

            # sum over l (mean /196 folded into w0) -> st_av -> avT
            av_f = pA.tile([BL, D], F32)
            st_av = pA.tile([16, D], BF16)
            nc.vector.memset(st_av[:], 0.0)
            for dch in range(4):
                sl = slice(dch * 512, (dch + 1) * 512)
                igc = pAs.tile([128, 2, 8, 512], BF16, tag="igA")
                nc.sync.dma_start(igc[:], inp["imgl"][:, :, :, sl])
                ps2 = psA.tile([128, 2, 512], F32, tag="psav")
                for r in range(2):
                    for g in range(4):
                        b = 2 * g + r
                        for lt in range(2):
                            mm(out=ps2[32 * g:32 * g + 1, r, :],
                               lhsT=ones128[:, 0:1],
                               rhs=igc[:, lt, b, :],
                               start=(lt == 0), stop=(lt == 1),
                               tile_position=(0, 32 * g))
                ev = pAs.tile([128, 2, 512], F32, tag="evA")
                act(ev[:], ps2[:], AF.Identity)
                nc.sync.dma_start(av_f[0:8:2, sl], ev[0:128:32, 0, :])
                nc.sync.dma_start(av_f[1:8:2, sl], ev[0:128:32, 1, :])
            nc.vector.tensor_copy(st_av[0:BL, :], av_f[:])
            avT = pA.tile([128, 16, 16], BF16)
            nc.sync.dma_start_transpose(avT[:], st_av[:])

            # h0 | c0 = tanh(sum_img @ (W0/196) + b0)
            for ch in range(8):
                sl = slice(ch * 256, (ch + 1) * 256)
                w0c = pAs.tile([128, 16, 256], BF16, tag="w0c")
                nc.sync.dma_start(w0c[:], inp["w0"][:, :, sl])
                ps = psA.tile([8, 256], F32, tag="ps0")
                for kt in range(16):
                    mm(out=ps[:], lhsT=avT[:, kt, 0:8], rhs=w0c[:, kt, :],
                       start=(kt == 0), stop=False)
                mm(out=ps[:], lhsT=ones1[0:1, 0:8], rhs=b0_s[0:1, sl],
                   start=False, stop=True)
                if ch < 4:
                    act(st_h[0:BL, sl], ps[:], AF.Tanh)
                else:
                    act(cst[:, slice((ch - 4) * 256, (ch - 3) * 256)],
                        ps[:], AF.Tanh)
            nc.sync.dma_start_transpose(hT[:, :, 0, :], st_h[:])

        # ================= Phase B: E_gates -> DRAM =================
        with tc.tile_pool(name="pB", bufs=1) as pB, \
             tc.tile_pool(name="pBs", bufs=2) as pBs, \
             tc.tile_pool(name="psB", bufs=2, space="PSUM") as psB:
            tid0 = pB.tile([128, 1], I32)
            nc.sync.dma_start(tid0[:], inp["tids"][0:128, :])
            tid1 = pB.tile([8, 1], I32)
            nc.sync.dma_start(tid1[:], inp["tids"][128:NR, :])
            g0 = pB.tile([128, EMB], BF16)
            nc.gpsimd.indirect_dma_start(
                out=g0[:], out_offset=None, in_=inp["embt"][:],
                in_offset=bass.IndirectOffsetOnAxis(ap=tid0[:, :1], axis=0))
            g1 = pB.tile([16, EMB], BF16)
            nc.vector.memset(g1[:], 0.0)
            nc.gpsimd.indirect_dma_start(
                out=g1[0:8, :], out_offset=None, in_=inp["embt"][:],
                in_offset=bass.IndirectOffsetOnAxis(ap=tid1[:, :1], axis=0))
            ET0 = pB.tile([128, 4, 128], BF16)
            nc.sync.dma_start_transpose(ET0[:], g0[:])
            ET1 = pB.tile([128, 4, 16], BF16)
            nc.sync.dma_start_transpose(ET1[:], g1[:])
            bg_s = pB.tile([1, 4 * RNN], BF16)
            nc.sync.dma_start(bg_s[:], inp["bg"][:])
            for nch in range(8):
                sl = slice(nch * 512, (nch + 1) * 512)
                wie_c = pBs.tile([128, 4, 512], BF16, tag="wie")
                nc.sync.dma_start(wie_c[:], inp["wie"][:, :, sl])
                ps = psB.tile([128, 512], F32, tag="psb0")
                for kt in range(4):
                    mm(out=ps[:], lhsT=ET0[:, kt, :], rhs=wie_c[:, kt, :],
                       start=(kt == 0), stop=False)
                mm(out=ps[:], lhsT=ones1[0:1, :], rhs=bg_s[0:1, sl],
                   start=False, stop=True)
                ecp = pBs.tile([128, 512], BF16, tag="ecp")
                nc.vector.tensor_copy(ecp[:], ps[:])
                nc.sync.dma_start(inp["egd"][0:128, sl], ecp[:])
                ps2 = psB.tile([8, 512], F32, tag="psb1")
                for kt in range(4):
                    mm(out=ps2[:], lhsT=ET1[:, kt, 0:8], rhs=wie_c[:, kt, :],
                       start=(kt == 0), stop=False)
                mm(out=ps2[:], lhsT=ones1[0:1, 0:8], rhs=bg_s[0:1, sl],
                   start=False, stop=True)
                ecp2 = pBs.tile([8, 512], BF16, tag="ecp2")
                nc.vector.tensor_copy(ecp2[:], ps2[:])
                nc.sync.dma_start(inp["egd"][128:NR, sl], ecp2[:])

        # ================= Phase C: the 17-step scan =================
        egv = inp["egd"].rearrange("r (g jc c) -> r g jc c", g=4, c=256)
        with tc.tile_pool(name="st2", bufs=1) as st2, \
             tc.tile_pool(name="pSufb", bufs=2) as pSufb, \
             tc.tile_pool(name="pSic", bufs=2) as pSic, \
             tc.tile_pool(name="pSig", bufs=2) as pSig, \
             tc.tile_pool(name="pAtt", bufs=2) as pAtt, \
             tc.tile_pool(name="pW", bufs=1) as pW, \
             tc.tile_pool(name="pT", bufs=2) as pT, \
             tc.tile_pool(name="psC", bufs=2, space="PSUM") as psC, \
             tc.tile_pool(name="psE", bufs=1, space="PSUM") as psE:
            whh_s = st2.tile([128, 8, 4 * RNN], BF16)
            nc.sync.dma_start(whh_s[:], inp["whh"][:])
            vv_s = st2.tile([128, 8], BF16)
            nc.sync.dma_start(vv_s[:], inp["vv"][:])
            bufb_s = st2.tile([1, 3 * RNN], BF16)
            nc.sync.dma_start(bufb_s[:], inp["bufb"][:])
            st_uh = st2.tile([16, RNN], BF16)
            st_xg = st2.tile([16, D], BF16)
            st_al = st2.tile([16, 256], BF16)
            nc.vector.memset(st_uh[:], 0.0)
            nc.vector.memset(st_xg[:], 0.0)
            nc.vector.memset(st_al[:], 0.0)

            for t in range(T1):
                # ---- U_h | gate = h @ [WU | Wfb] + [0 | bfb] ----
                gate_b = pW.tile([BL, D], BF16, tag="gate")
                for ch in range(6):
                    sl = slice(ch * 512, (ch + 1) * 512)
                    wch = pSufb.tile([128, 8, 512], BF16, tag="wufb")
                    nc.sync.dma_start(wch[:], inp["wufb"][:, :, sl])
                    ps = psC.tile([8, 512], F32, tag="psug")
                    for kt in range(8):
                        mm(out=ps[:], lhsT=hT[:, kt, t, 0:8], rhs=wch[:, kt, :],
                           start=(kt == 0), stop=False)
                    mm(out=ps[:], lhsT=ones1[0:1, 0:8], rhs=bufb_s[0:1, sl],
                       start=False, stop=True)
                    if ch < 2:
                        act(st_uh[0:BL, sl], ps[:], AF.Identity)
                    else:
                        act(gate_b[:, slice((ch - 2) * 512, (ch - 1) * 512)],
                            ps[:], AF.Sigmoid)
                uhT = pT.tile([128, 8, 16], BF16, tag="uhT")
                nc.sync.dma_start_transpose(uhT[:], st_uh[:])

                # ---- attention: att = tanh(Ws + Uh); e = v . att ----
                pse = psE.tile([128, 392], F32, tag="pse")
                for fc in range(8):
                    attb = pAtt.tile([128, BLL], BF16, tag="att")
                    u = uhT[:, fc, 0:8]
                    ub = bass.AP(tensor=u.tensor, offset=u.offset,
                                 ap=list(u.ap) + [[0, L]])
                    nc.vector.tensor_add(attb[:], WsT[:, fc, :], ub)
                    act(attb[:], attb[:], AF.Tanh)
                    for g in range(4):
                        mm(out=pse[32 * g:32 * g + 1, :],
                           lhsT=vv_s[:, fc:fc + 1],
                           rhs=attb[:, 392 * g:392 * (g + 1)],
                           start=(fc == 0), stop=(fc == 7),
                           tile_position=(0, 32 * g))

                # ---- softmax over l ----
                ev = pW.tile([128, 392], F32, tag="ev")
                act(ev[:], pse[:], AF.Identity)
                e8 = pW.tile([BL, L], F32, tag="e8")
                nc.sync.dma_start(e8[0:8:2, :], ev[0:128:32, 0:196])
                nc.sync.dma_start(e8[1:8:2, :], ev[0:128:32, 196:392])
                nmax = pW.tile([BL, 1], F32, tag="nmax")
                nc.vector.reduce_max(nmax[:], e8[:], axis=X, negate=True)
                ex = pW.tile([BL, L], F32, tag="ex")
                act(ex[:], e8[:], AF.Exp, bias=nmax[:, 0:1])
                ssum = pW.tile([BL, 1], F32, tag="ssum")
                nc.vector.reduce_sum(ssum[:], ex[:], axis=X)
                rs = pW.tile([BL, 1], F32, tag="rs")
                nc.vector.reciprocal(rs[:], ssum[:])
                al = pW.tile([BL, L], F32, tag="al")
                nc.vector.tensor_scalar_mul(al[:], ex[:], rs[:, 0:1])
                nc.sync.dma_start(alph[8 * t:8 * t + 8, :], al[:])
                nc.vector.tensor_copy(st_al[0:BL, 0:L], al[:])
                alT = pT.tile([128, 2, 16], BF16, tag="alT")
                nc.sync.dma_start_transpose(alT[:], st_al[:])

                # ---- context = sum_l alpha * img (img streamed) ----
                ctx_b = pW.tile([BL, D], BF16, tag="ctx")
                for dch in range(8):
                    sl = slice(dch * 256, (dch + 1) * 256)
                    igc = pSig.tile([128, 2, 8, 256], BF16, tag="ig")
                    nc.sync.dma_start(igc[:], inp["imgl"][:, :, :, sl])
                    psc = psC.tile([128, 2, 256], F32, tag="psc")
                    for r in range(2):
                        for g in range(4):
                            b = 2 * g + r
                            for lt in range(2):
                                mm(out=psc[32 * g:32 * g + 1, r, :],
                                   lhsT=alT[:, lt, b:b + 1],
                                   rhs=igc[:, lt, b, :],
                                   start=(lt == 0), stop=(lt == 1),
                                   tile_position=(0, 32 * g))
                    evc = pW.tile([128, 2, 256], BF16, tag="evc")
                    act(evc[:], psc[:], AF.Identity)
                    nc.sync.dma_start(ctx_b[0:8:2, sl], evc[0:128:32, 0, :])
                    nc.sync.dma_start(ctx_b[1:8:2, sl], evc[0:128:32, 1, :])

                # ---- x_ctx = gate * context ----
                nc.vector.tensor_mul(st_xg[0:BL, :], gate_b[:], ctx_b[:])
                xgT = pT.tile([128, 16, 16], BF16, tag="xgT")
                nc.sync.dma_start_transpose(xgT[:], st_xg[:])

                # ---- gates + LSTM cell, chunked by 256-feature block ----
                for jc in range(4):
                    egc = pW.tile([8, 4, 256], BF16, tag="egc")
                    nc.sync.dma_start(egc[:], egv[8 * t:8 * t + 8, :, jc, :])
                    sg = []
                    for g in range(4):
                        sl = slice(g * RNN + jc * 256, g * RNN + jc * 256 + 256)
                        wcic = pSic.tile([128, 16, 256], BF16, tag="wic")
                        nc.sync.dma_start(wcic[:], inp["wic"][:, :, sl])
                        ps = psC.tile([8, 256], F32, tag="psg")
                        for kt in range(8):
                            mm(out=ps[:], lhsT=hT[:, kt, t, 0:8],
                               rhs=whh_s[:, kt, sl], start=(kt == 0), stop=False)
                        for kt in range(16):
                            mm(out=ps[:], lhsT=xgT[:, kt, 0:8],
                               rhs=wcic[:, kt, :], start=False, stop=False)
                        mm(out=ps[:], lhsT=i8_s[:], rhs=egc[:, g, :],
                           start=False, stop=True)
                        sgt = pW.tile([BL, 256], F32, tag=f"sg{g}")
                        act(sgt[:], ps[:], AF.Tanh if g == 2 else AF.Sigmoid)
                        sg.append(sgt)
                    csl = slice(jc * 256, (jc + 1) * 256)
                    t1_ = pW.tile([BL, 256], F32, tag="t1")
                    nc.vector.tensor_mul(t1_[:], sg[1][:], cst[:, csl])
                    t2_ = pW.tile([BL, 256], F32, tag="t2")
                    nc.vector.tensor_mul(t2_[:], sg[0][:], sg[2][:])
                    nc.vector.tensor_add(cst[:, csl], t1_[:], t2_[:])
                    tcc = pW.tile([BL, 256], F32, tag="tcc")
                    act(tcc[:], cst[:, csl], AF.Tanh)
                    nc.vector.tensor_mul(st_h[0:BL, csl], sg[3][:], tcc[:])
                nc.sync.dma_start_transpose(hT[:, :, t + 1, :], st_h[:])

        # ================= Phase D: preds = H @ Wout + bout =================
        with tc.tile_pool(name="pD", bufs=3) as pD, \
             tc.tile_pool(name="pD1", bufs=1) as pD1, \
             tc.tile_pool(name="psD", bufs=2, space="PSUM") as psD:
            # repack h history: hT2[:, kt, t*8+b] = h_{t+1}[b, kt*128+p]
            hT2 = pD1.tile([128, 8, NR], BF16)
            for kt in range(8):
                nc.sync.dma_start(hT2[:, kt, :], hT[:, kt, 1:18, 0:8])
            nchs = [(i * 512, 512) for i in range(19)] + [(19 * 512, VOUT - 19 * 512)]
            for off, w in nchs:
                wo = pD.tile([128, 8, 512], BF16, tag="wo")
                nc.sync.dma_start(wo[:, :, 0:w], inp["wout"][:, :, off:off + w])
                bo = pD.tile([1, 512], BF16, tag="bo")
                nc.sync.dma_start(bo[0:1, 0:w], inp["bout"][0:1, off:off + w])
                for mc in range(2):
                    mrows = 128 if mc == 0 else 8
                    ps = psD.tile([128, 512], F32, tag="pso")
                    for kt in range(8):
                        lhsT = (hT2[:, kt, 0:128] if mc == 0
                                else hT2[:, kt, 128:NR])
                        mm(out=ps[0:mrows, 0:w], lhsT=lhsT, rhs=wo[:, kt, 0:w],
                           start=(kt == 0), stop=False)
                    mm(out=ps[0:mrows, 0:w], lhsT=ones1[0:1, 0:mrows],
                       rhs=bo[0:1, 0:w], start=False, stop=True)
                    po = pD.tile([128, 512], F32, tag="po")
                    act(po[0:mrows, 0:w], ps[0:mrows, 0:w], AF.Identity)
                    nc.sync.dma_start(preds[mc * 128:mc * 128 + mrows, off:off + w],
                                      po[0:mrows, 0:w])


def _bf(a):
    return np.ascontiguousarray(np.asarray(a, np.float32)).astype(ml_dtypes.bfloat16)


def _k_on_p(w, nkt):
    """[K, N] -> [128, nkt, N] with k = kt*128 + p."""
    K, N = w.shape
    assert K == nkt * 128
    return np.ascontiguousarray(np.transpose(w.reshape(nkt, 128, N), (1, 0, 2)))


def kernel(img_features, captions, emb, Wih, Whh, bih, bhh, Wh0, bh0, Wc0, bc0,
           Wfb, bfb, Wout, bout, WU, bU, WW, bW, Wv, bv):
    img_features = np.asarray(img_features, np.float32)
    captions = np.asarray(captions)
    f32 = lambda a: np.asarray(a, np.float32)

    if "nc" not in _cache:
        _cache["nc"] = _build_nc()
    nc = _cache["nc"]

    # ---- shared (replicated) weight prep, host side: dtype/layout only ----
    shared = {}
    shared["ww"] = _k_on_p(_bf(WW), 16)
    shared["bwu"] = np.ascontiguousarray(
        (f32(bW) + f32(bU)).reshape(8, 128).T).astype(np.float32)
    w0 = np.concatenate([f32(Wh0), f32(Wc0)], axis=1) / float(L)
    shared["w0"] = _k_on_p(_bf(w0), 16)
    shared["b0"] = _bf(np.concatenate([f32(bh0), f32(bc0)])).reshape(1, 2 * RNN)
    shared["embt"] = _bf(emb)
    shared["wie"] = _k_on_p(_bf(f32(Wih)[:EMB]), 4)
    shared["bg"] = _bf(f32(bih) + f32(bhh)).reshape(1, 4 * RNN)
    shared["whh"] = _k_on_p(_bf(Whh), 8)
    shared["vv"] = np.ascontiguousarray(
        _bf(Wv).reshape(8, 128).T)                      # v[kt*128+p] at [p, kt]
    wufb = np.concatenate([f32(WU), f32(Wfb)], axis=1)
    shared["wufb"] = _k_on_p(_bf(wufb), 8)
    shared["bufb"] = _bf(np.concatenate([np.zeros(RNN, np.float32), f32(bfb)])
                         ).reshape(1, 3 * RNN)
    shared["wic"] = _k_on_p(_bf(f32(Wih)[EMB:]), 16)
    shared["wout"] = _k_on_p(_bf(Wout), 8)
    shared["bout"] = _bf(bout).reshape(1, VOUT)
    shared["i8"] = np.eye(8, dtype=ml_dtypes.bfloat16)

    # target ids: prepend START token
    tgt = np.concatenate(
        [np.full((B, 1), START, np.int64), np.asarray(captions, np.int64)],
        axis=1)  # [64, 17]

    in_maps = []
    for c in range(N_CORES):
        rows = slice(c * BL, (c + 1) * BL)
        m = dict(shared)
        imgb = _bf(img_features[rows])                   # [8, 196, 2048]
        it = np.transpose(imgb, (2, 0, 1)).reshape(16, 128, BLL)
        m["imgT"] = np.ascontiguousarray(np.transpose(it, (1, 0, 2)))
        pad = np.zeros((BL, 256, D), ml_dtypes.bfloat16)
        pad[:, :L, :] = imgb
        m["imgl"] = np.ascontiguousarray(
            np.transpose(pad.reshape(BL, 2, 128, D), (2, 1, 0, 3)))
        m["tids"] = np.ascontiguousarray(
            tgt[rows].T.reshape(NR, 1)).astype(np.int32)
        in_maps.append(m)

    import time as _time
    _t0 = _time.perf_counter()
    res = run_bass_kernel_spmd(nc, in_maps, core_ids=list(range(N_CORES)))
    _cache["exec_wall_s"] = _time.perf_counter() - _t0
    _cache["last_res"] = res

    preds = np.zeros((B, T1, VOUT), np.float32)
    alphas = np.zeros((B, T1, L), np.float32)
    for c in range(N_CORES):
        r = res.results[c]
        preds[c * BL:(c + 1) * BL] = np.transpose(
            r["preds"].reshape(T1, BL, VOUT), (1, 0, 2))
        alphas[c * BL:(c + 1) * BL] = np.transpose(
            r["alph"].reshape(T1, BL, L), (1, 0, 2))
    return preds, alphas
